# revision 45
# baseline (speedup 1.0000x reference)
"""Hanning template layer for TRN2: weighted sum of 4 Hanning correlations
== single 80-tap correlation.  out[b,j] = sum_i c[i] * x[b, j+i-40].

Device scheme (per core, 8 batch rows of L=65536, pure data parallel):
  Host ships x as fp16 in "transposed" layout x[k, r*528 + v] =
  xrow_r[128 v + k], each row padded to 528*128 with a 64-sample zero
  lead-in (the lead-in centers the 80-tap window so TWO shifted matmuls,
  not three, cover every output block).
  1. Pair-granular DMA loads on both HWDGE queues.  HWDGE feeds the 16
     SDMA engines at ~1 descriptor / 9 ns, so DMA bandwidth scales with
     descriptor size; a row pair = 2112 B contiguous per partition.
  2. conv as 2 matmuls per row, band STATIONARY, signal MOVING (N=512):
       OT[m, n] = y[128 n + m] = sum_{s=0,1} sum_k Bs[s][k,m] xt[k, n+s]
       Bs[s][k, m] = c[128 s + k - m - 24]  (banded Toeplitz, fp16)
     accumulated in one f32 PSUM bank per row.  8 dummy warm-up matmuls
     on zeros run while the first rows are in flight, flipping the PE
     HAM clock gate to 2.4 GHz before the real stream.  Row pairs are
     consumed in DMA-arrival order (01, 45, 23, 67).
  3. DVE/ACT cast-copy PSUM->SBUF fp16 (final pair: one row per engine,
     in parallel); one 256 KiB store per row pair (2 KiB runs per
     partition) -- mid stores on the sync queue, the final store on the
     otherwise-idle scalar queue so its descriptor feed starts at once.
  Host un-shuffles the [pair, m, o, n] output (one cheap np transpose).

Constraints baked in (learned on HW):
  - walrus codegen allows only ONE sync wait per instruction -> a post-
    pass splits residual multi-waits onto cloned per-engine Drains.
  - each dma_start costs ~0.7 us of issuing-sequencer occupancy and
    ~2 us of HBM completion latency; keep DMA count low, sizes big.
  - matmul emits LDWEIGHTS per call; N=512 fp16 moving operand streams
    at ~216 ns warm (2.4 GHz), ~427 ns cold (1.2 GHz).
"""

import copy as _copy

import numpy as np

import concourse.bass as bass
import concourse.mybir as mybir
from concourse.tile import TileContext
from concourse.bass_utils import run_bass_kernel_spmd

B, L = 64, 65536
N_CORES = 8
ROWS = B // N_CORES          # 8 rows per core
P = 128                      # partitions / block size
NBLK = L // P                # 512 blocks of 128 samples per row
VB = 528                     # padded blocks per row (mult of 16 for xbar)
OFF = 64                     # zero lead-in samples (centers the window)
TAPS = 80
HALF = 40
NSH = 2                      # shifted matmuls per output chunk
NCH = 4                      # output chunks of 128 blocks per row

F32 = mybir.dt.float32
F16 = mybir.dt.float16

WIDTHS = [10, 20, 30, 40]


def _combined_filter(template_weights: np.ndarray) -> np.ndarray:
    """softmax-weighted sum of hanning(2w) templates aligned at offset d=-40."""
    w = template_weights.astype(np.float64)
    e = np.exp(w - w.max())
    sm = e / e.sum()
    c = np.zeros(TAPS, dtype=np.float64)
    for t, wd in enumerate(WIDTHS):
        h = np.hanning(2 * wd)
        # contributes at filter index i = d + 40 for d in [-wd, wd)
        c[HALF - wd : HALF + wd] += sm[t] * h
    return c


def _band_matrices(c: np.ndarray) -> np.ndarray:
    """Bs[s][k, m] = c[128 s + k - m - 24] where in range, else 0."""
    Bs = np.zeros((NSH, P, P), dtype=np.float64)
    k = np.arange(P)[:, None]
    m = np.arange(P)[None, :]
    for s in range(NSH):
        i = 128 * s + k - m - 24
        ok = (i >= 0) & (i < TAPS)
        Bs[s][ok] = c[i[ok]]
    return Bs


def _split_excess_waits(nc, limit=1):
    """Move excess sync waits onto cloned same-engine Drain instructions
    (walrus codegen rejects >1 wait per instruction)."""
    drain_tmpl = {}
    for func in nc.m.functions:
        for bb in func.blocks:
            for inst in bb.instructions:
                if inst.opcode == "Drain" and inst.engine not in drain_tmpl:
                    drain_tmpl[inst.engine] = inst
    for func in nc.m.functions:
        for bb in func.blocks:
            changed = False
            out = []
            for inst in bb.instructions:
                si = inst.sync_info
                if si and len(si.on_wait) > limit:
                    waits = list(si.on_wait)
                    keep, extra = waits[-limit:], waits[:-limit]
                    tmpl = inst if inst.opcode == "Drain" else drain_tmpl.get(inst.engine)
                    assert tmpl is not None, (
                        f"no drain template for engine {inst.engine} ({inst.opcode})"
                    )
                    for j in range(0, len(extra), limit):
                        cln = _copy.deepcopy(tmpl)
                        cln.name = f"{inst.name}w{j}"
                        cln.engine = inst.engine
                        csi = cln.sync_info
                        csi.on_wait = extra[j : j + limit]
                        csi.on_update = []
                        cln.sync_info = csi
                        out.append(cln)
                        changed = True
                    si.on_wait = keep
                    inst.sync_info = si
                out.append(inst)
            if changed:
                bb.instructions = out


def build_nc():
    nc = bass.Bass()
    # pre-transposed on host: x[k, r*VB + v] = xrow_r[128 v + k].
    # Row-grouped per partition -> 4224 B contiguous per partition per
    # 4-row load (the HWDGE descriptor feed is ~9 ns/desc, so descriptor
    # SIZE sets DMA bandwidth: 1 KiB descs cap at ~115 GB/s aggregate).
    x = nc.dram_tensor("x", [P, ROWS * VB], F16, kind="ExternalInput")
    consts = nc.dram_tensor("consts", [P, NSH * P], F16, kind="ExternalInput")
    # transposed, pair-grouped output: y[p, m, o, n] = y_nat[2p+o, 128 n + m]
    # -> 2 KiB contiguous per partition per pair-store; host un-shuffles.
    y = nc.dram_tensor("y", [ROWS // 2, P, 2, NBLK], F16, kind="ExternalOutput")

    with TileContext(nc) as tc:
        with (
            tc.tile_pool(name="sbuf", bufs=ROWS) as pool,
            tc.tile_pool(name="opool", bufs=4) as opool,
            tc.tile_pool(name="cpool", bufs=1) as cpool,
            tc.tile_pool(name="psum", bufs=6, space="PSUM") as pp,
            tc.tile_pool(name="wpsum", bufs=1, space="PSUM") as wp,
        ):
            # row-pair loads: sync queue takes rows 0-3 (first compute),
            # scalar queue takes consts + rows 4-7.  Pair granularity gets
            # the first matmul started ~2 us sooner than one 4-row load.
            cst = cpool.tile([P, NSH * P], F16)
            # Loads: descriptor-feed rate scales with run length (2112 B
            # pairs ~230 GB/s/queue, 4224 B quads ~line rate).  Sync takes
            # pairs 01 and 23 (early start); scalar takes consts then rows
            # 4-7 as one line-rate load.  Arrival order: 01, 4567, 23.
            xts = [
                pool.tile([P, 2 * VB], F16, tag="xt", name=f"xt{g}")
                for g in range(4)
            ]
            nc.sync.dma_start(out=xts[0], in_=x[:, 0 : 2 * VB])
            nc.scalar.dma_start(out=cst, in_=consts[:, :])
            nc.sync.dma_start(out=xts[1], in_=x[:, 2 * VB : 4 * VB])
            nc.scalar.dma_start(out=xts[2], in_=x[:, 4 * VB : 6 * VB])
            nc.scalar.dma_start(out=xts[3], in_=x[:, 6 * VB : 8 * VB])

            def row_slice(r, ncols):
                return xts[r // 2][:, (r % 2) * VB : (r % 2) * VB + ncols]

            # HAM warm-up: dummy matmuls on zeros while the first x rows are
            # still in flight (PE would idle; this flips the clock gate to
            # 8/8 so the real matmuls run at 2.4 GHz).  GpSimd memset: DVE's
            # own memset starts ~0.5 us later (it is otherwise busy-free but
            # sits behind the slower engine preamble).
            wtile = cpool.tile([P, NBLK], F16)
            nc.gpsimd.memset(wtile, 0.0)
            ps_w = wp.tile([P, NBLK], F32)
            for _ in range(8):
                nc.tensor.matmul(ps_w, wtile[:, 0:P], wtile, start=True, stop=True)

            # consume pairs in DMA-arrival order (queues fill concurrently)
            PAIR_ORDER = [0, 2, 1, 3]
            for idx, pair in enumerate(PAIR_ORDER):
                last = idx == len(PAIR_ORDER) - 1
                osb = opool.tile([P, 2 * NBLK], F16, tag="osb")
                for o in range(2):
                    r = 2 * pair + o
                    # OT[m, n] = y[128 n + m] = sum_s sum_k Bs[s][k,m] xt'[k,n+s]
                    ps = pp.tile([P, NBLK], F32, tag="ps")
                    for s in range(NSH):
                        nc.tensor.matmul(
                            ps,
                            cst[:, P * s : P * (s + 1)],
                            row_slice(r, NBLK + s + 1)[:, s : s + NBLK],
                            start=(s == 0),
                            stop=(s == NSH - 1),
                        )
                    dst = osb[:, o * NBLK : (o + 1) * NBLK]
                    if last:
                        # final pair: one row per engine, copies in parallel
                        if o == 0:
                            nc.scalar.copy(out=dst, in_=ps)
                        else:
                            nc.vector.tensor_copy(out=dst, in_=ps)
                    elif o == 0:
                        nc.vector.tensor_copy(out=dst, in_=ps)
                    else:
                        nc.scalar.copy(out=dst, in_=ps)
                # one store per row pair (2 KiB contiguous per partition).
                # Mid stores all on sync; the final store gets the idle
                # scalar queue so its descriptor feed starts immediately.
                st_eng = nc.scalar if last else nc.sync
                st_eng.dma_start(
                    out=y[pair],
                    in_=osb.rearrange("m (o n) -> m o n", o=2),
                )

    _split_excess_waits(nc)
    return nc


_NC_CACHE = None


def _host_prep(x: np.ndarray, template_weights: np.ndarray):
    """fp16 cast + pad + block-transpose of x rows; fp16 band-matrix consts."""
    c = _combined_filter(np.asarray(template_weights, dtype=np.float32))
    Bs = _band_matrices(c)
    consts = np.concatenate(list(Bs), axis=1).astype(np.float16)

    xpad = np.zeros((B, VB * P), dtype=np.float16)
    xpad[:, OFF : OFF + L] = np.asarray(x, dtype=np.float32)
    # per core: x_t[k, r*VB + v] = xrow_r[128 v + k]
    x_t = np.ascontiguousarray(
        xpad.reshape(N_CORES, ROWS, VB, P).transpose(0, 3, 1, 2)
    ).reshape(N_CORES, P, ROWS * VB)
    return x_t, consts


def kernel(x: np.ndarray, template_weights: np.ndarray) -> np.ndarray:
    global _NC_CACHE
    xpad, consts = _host_prep(x, template_weights)

    if _NC_CACHE is None:
        _NC_CACHE = build_nc()
    nc = _NC_CACHE

    in_maps = [
        {"x": xpad[core], "consts": consts} for core in range(N_CORES)
    ]
    res = run_bass_kernel_spmd(nc, in_maps, core_ids=list(range(N_CORES)))
    out = np.stack([r["y"] for r in res.results], axis=0)  # [C, 4, P, 2, NBLK]
    # un-shuffle: y_nat[8c + 2p + o, 128 n + m] = out[c, p, m, o, n]
    return np.ascontiguousarray(
        out.astype(np.float32).transpose(0, 1, 3, 4, 2)
    ).reshape(B, L)


# revision 47
# speedup vs baseline: 1.1800x; 1.1800x over previous
"""Hanning template layer for TRN2: weighted sum of 4 Hanning correlations
== single 80-tap correlation.  out[b,j] = sum_i c[i] * x[b, j+i-40].

Device scheme (per core, 8 batch rows of L=65536, pure data parallel):
  Host ships x as fp16 in "transposed" layout x[k, r*528 + v] =
  xrow_r[128 v + k], each row padded to 528*128 with a 64-sample zero
  lead-in (the lead-in centers the 80-tap window so TWO shifted matmuls,
  not three, cover every output block).
  1. Pair-granular DMA loads on both HWDGE queues.  HWDGE feeds the 16
     SDMA engines at ~1 descriptor / 9 ns, so DMA bandwidth scales with
     descriptor size; a row pair = 2112 B contiguous per partition.
  2. conv as 2 matmuls per row, band STATIONARY, signal MOVING (N=512):
       OT[m, n] = y[128 n + m] = sum_{s=0,1} sum_k Bs[s][k,m] xt[k, n+s]
       Bs[s][k, m] = c[128 s + k - m - 24]  (banded Toeplitz, fp16)
     accumulated in one f32 PSUM bank per row.  8 dummy warm-up matmuls
     on zeros run while the first rows are in flight, flipping the PE
     HAM clock gate to 2.4 GHz before the real stream.  The consts ride
     at the head of the first sync load; row pairs are consumed in
     DMA-arrival order across the two queues (01, 45, 23, 67).
  3. DVE/ACT cast-copy PSUM->SBUF fp16 (final pair: one row per engine,
     in parallel); one 256 KiB store per row pair (2 KiB runs per
     partition) -- mid stores on the sync queue, the final store on the
     otherwise-idle scalar queue so its descriptor feed starts at once.
  Host un-shuffles the [pair, m, o, n] output (one cheap np transpose).

Constraints baked in (learned on HW):
  - walrus codegen allows only ONE sync wait per instruction -> a post-
    pass splits residual multi-waits onto cloned per-engine Drains.
  - each dma_start costs ~0.7 us of issuing-sequencer occupancy and
    ~2 us of HBM completion latency; keep DMA count low, sizes big.
  - matmul emits LDWEIGHTS per call; N=512 fp16 moving operand streams
    at ~216 ns warm (2.4 GHz), ~427 ns cold (1.2 GHz).
"""

import copy as _copy

import numpy as np

import concourse.bass as bass
import concourse.mybir as mybir
from concourse.tile import TileContext
from concourse.bass_utils import run_bass_kernel_spmd

B, L = 64, 65536
N_CORES = 8
ROWS = B // N_CORES          # 8 rows per core
P = 128                      # partitions / block size
NBLK = L // P                # 512 blocks of 128 samples per row
VB = 528                     # padded blocks per row (mult of 16 for xbar)
OFF = 64                     # zero lead-in samples (centers the window)
TAPS = 80
HALF = 40
NSH = 2                      # shifted matmuls per output chunk
NCH = 4                      # output chunks of 128 blocks per row

F32 = mybir.dt.float32
F16 = mybir.dt.float16

WIDTHS = [10, 20, 30, 40]


def _combined_filter(template_weights: np.ndarray) -> np.ndarray:
    """softmax-weighted sum of hanning(2w) templates aligned at offset d=-40."""
    w = template_weights.astype(np.float64)
    e = np.exp(w - w.max())
    sm = e / e.sum()
    c = np.zeros(TAPS, dtype=np.float64)
    for t, wd in enumerate(WIDTHS):
        h = np.hanning(2 * wd)
        # contributes at filter index i = d + 40 for d in [-wd, wd)
        c[HALF - wd : HALF + wd] += sm[t] * h
    return c


def _band_matrices(c: np.ndarray) -> np.ndarray:
    """Bs[s][k, m] = c[128 s + k - m - 24] where in range, else 0."""
    Bs = np.zeros((NSH, P, P), dtype=np.float64)
    k = np.arange(P)[:, None]
    m = np.arange(P)[None, :]
    for s in range(NSH):
        i = 128 * s + k - m - 24
        ok = (i >= 0) & (i < TAPS)
        Bs[s][ok] = c[i[ok]]
    return Bs


def _split_excess_waits(nc, limit=1):
    """Move excess sync waits onto cloned same-engine Drain instructions
    (walrus codegen rejects >1 wait per instruction)."""
    drain_tmpl = {}
    for func in nc.m.functions:
        for bb in func.blocks:
            for inst in bb.instructions:
                if inst.opcode == "Drain" and inst.engine not in drain_tmpl:
                    drain_tmpl[inst.engine] = inst
    for func in nc.m.functions:
        for bb in func.blocks:
            changed = False
            out = []
            for inst in bb.instructions:
                si = inst.sync_info
                if si and len(si.on_wait) > limit:
                    waits = list(si.on_wait)
                    keep, extra = waits[-limit:], waits[:-limit]
                    tmpl = inst if inst.opcode == "Drain" else drain_tmpl.get(inst.engine)
                    assert tmpl is not None, (
                        f"no drain template for engine {inst.engine} ({inst.opcode})"
                    )
                    for j in range(0, len(extra), limit):
                        cln = _copy.deepcopy(tmpl)
                        cln.name = f"{inst.name}w{j}"
                        cln.engine = inst.engine
                        csi = cln.sync_info
                        csi.on_wait = extra[j : j + limit]
                        csi.on_update = []
                        cln.sync_info = csi
                        out.append(cln)
                        changed = True
                    si.on_wait = keep
                    inst.sync_info = si
                out.append(inst)
            if changed:
                bb.instructions = out


def build_nc():
    nc = bass.Bass()
    # pre-transposed on host: x[k, r*VB + v] = xrow_r[128 v + k].
    # Row-grouped per partition -> 4224 B contiguous per partition per
    # 4-row load (the HWDGE descriptor feed is ~9 ns/desc, so descriptor
    # SIZE sets DMA bandwidth: 1 KiB descs cap at ~115 GB/s aggregate).
    # consts (NSH*P cols) packed ahead of the row data so the first sync
    # load delivers them with rows 0-1 and the scalar queue starts on row
    # data immediately (~0.7 us earlier arrival for rows 4-5)
    x = nc.dram_tensor("x", [P, NSH * P + ROWS * VB], F16, kind="ExternalInput")
    # transposed, pair-grouped output: y[p, m, o, n] = y_nat[2p+o, 128 n + m]
    # -> 2 KiB contiguous per partition per pair-store; host un-shuffles.
    y = nc.dram_tensor("y", [ROWS // 2, P, 2, NBLK], F16, kind="ExternalOutput")

    with TileContext(nc) as tc:
        with (
            tc.tile_pool(name="sbuf", bufs=ROWS) as pool,
            tc.tile_pool(name="opool", bufs=4) as opool,
            tc.tile_pool(name="cpool", bufs=1) as cpool,
            tc.tile_pool(name="psum", bufs=6, space="PSUM") as pp,
            tc.tile_pool(name="wpsum", bufs=1, space="PSUM") as wp,
        ):
            # Loads: descriptor-feed rate scales with run length (2112 B
            # pairs ~230 GB/s/queue).  Sync takes consts+rows01 then rows23;
            # scalar takes rows45 then rows67 (its first slot is no longer
            # burnt on the consts transfer).
            CW = NSH * P
            xt0c = pool.tile([P, CW + 2 * VB], F16, tag="xt0")
            xts = [
                pool.tile([P, 2 * VB], F16, tag="xt", name=f"xt{g}")
                for g in range(3)
            ]
            cst = xt0c[:, 0:CW]
            nc.sync.dma_start(out=xt0c, in_=x[:, 0 : CW + 2 * VB])
            nc.scalar.dma_start(out=xts[1], in_=x[:, CW + 4 * VB : CW + 6 * VB])
            nc.sync.dma_start(out=xts[0], in_=x[:, CW + 2 * VB : CW + 4 * VB])
            nc.scalar.dma_start(out=xts[2], in_=x[:, CW + 6 * VB : CW + 8 * VB])

            def row_slice(r, ncols):
                if r < 2:
                    return xt0c[:, CW + r * VB : CW + r * VB + ncols]
                g, o = (r - 2) // 2, (r - 2) % 2
                return xts[g][:, o * VB : o * VB + ncols]

            # HAM warm-up: dummy matmuls on zeros while the first x rows are
            # still in flight (PE would idle; this flips the clock gate to
            # 8/8 so the real matmuls run at 2.4 GHz).  GpSimd memset: DVE's
            # own memset starts ~0.5 us later (it is otherwise busy-free but
            # sits behind the slower engine preamble).
            wtile = cpool.tile([P, NBLK], F16)
            nc.gpsimd.memset(wtile, 0.0)
            ps_w = wp.tile([P, NBLK], F32)
            for _ in range(8):
                nc.tensor.matmul(ps_w, wtile[:, 0:P], wtile, start=True, stop=True)

            # consume pairs in DMA-arrival order (queues fill concurrently)
            PAIR_ORDER = [0, 2, 1, 3]
            for idx, pair in enumerate(PAIR_ORDER):
                last = idx == len(PAIR_ORDER) - 1
                osb = opool.tile([P, 2 * NBLK], F16, tag="osb")
                for o in range(2):
                    r = 2 * pair + o
                    # OT[m, n] = y[128 n + m] = sum_s sum_k Bs[s][k,m] xt'[k,n+s]
                    ps = pp.tile([P, NBLK], F32, tag="ps")
                    for s in range(NSH):
                        nc.tensor.matmul(
                            ps,
                            cst[:, P * s : P * (s + 1)],
                            row_slice(r, NBLK + s + 1)[:, s : s + NBLK],
                            start=(s == 0),
                            stop=(s == NSH - 1),
                        )
                    dst = osb[:, o * NBLK : (o + 1) * NBLK]
                    if last:
                        # final pair: one row per engine, copies in parallel
                        if o == 0:
                            nc.scalar.copy(out=dst, in_=ps)
                        else:
                            nc.vector.tensor_copy(out=dst, in_=ps)
                    elif o == 0:
                        nc.vector.tensor_copy(out=dst, in_=ps)
                    else:
                        nc.scalar.copy(out=dst, in_=ps)
                # one store per row pair (2 KiB contiguous per partition).
                # Mid stores all on sync; the final store gets the idle
                # scalar queue so its descriptor feed starts immediately.
                st_eng = nc.scalar if last else nc.sync
                st_eng.dma_start(
                    out=y[pair],
                    in_=osb.rearrange("m (o n) -> m o n", o=2),
                )

    _split_excess_waits(nc)
    return nc


_NC_CACHE = None


def _host_prep(x: np.ndarray, template_weights: np.ndarray):
    """fp16 cast + pad + block-transpose of x rows; fp16 band-matrix consts."""
    c = _combined_filter(np.asarray(template_weights, dtype=np.float32))
    Bs = _band_matrices(c)
    consts = np.concatenate(list(Bs), axis=1).astype(np.float16)

    xpad = np.zeros((B, VB * P), dtype=np.float16)
    xpad[:, OFF : OFF + L] = np.asarray(x, dtype=np.float32)
    # per core: [consts | x_t] with x_t[k, r*VB + v] = xrow_r[128 v + k]
    x_t = np.ascontiguousarray(
        xpad.reshape(N_CORES, ROWS, VB, P).transpose(0, 3, 1, 2)
    ).reshape(N_CORES, P, ROWS * VB)
    packed = np.concatenate(
        [np.broadcast_to(consts, (N_CORES, P, NSH * P)), x_t], axis=2
    )
    return np.ascontiguousarray(packed), None


def kernel(x: np.ndarray, template_weights: np.ndarray) -> np.ndarray:
    global _NC_CACHE
    xpad, consts = _host_prep(x, template_weights)

    if _NC_CACHE is None:
        _NC_CACHE = build_nc()
    nc = _NC_CACHE

    in_maps = [{"x": xpad[core]} for core in range(N_CORES)]
    res = run_bass_kernel_spmd(nc, in_maps, core_ids=list(range(N_CORES)))
    out = np.stack([r["y"] for r in res.results], axis=0)  # [C, 4, P, 2, NBLK]
    # un-shuffle: y_nat[8c + 2p + o, 128 n + m] = out[c, p, m, o, n]
    return np.ascontiguousarray(
        out.astype(np.float32).transpose(0, 1, 3, 4, 2)
    ).reshape(B, L)
